# revision 9
# baseline (speedup 1.0000x reference)
"""TRN2 Bass kernel for nn_CNN_transformer_hr_xyz_41051297415299.

Reference model (B=32, C=512, D=512, H=8, DFF=2048, K=7), per batch element:
    query_in = causal_conv_in(x)                 # conv over last axis t, mixing C
    xn       = LN0(query_in)                     # over t, (x-m)/(std+eps), ddof=1
    q = conv_q(query_in); k = conv_k(xn); v = conv_v(xn)
    heads split the t axis (8 x 64); attention over the C axis
    o  = softmax(q k^T / 8) v   -> (C, D)
    y  = conv_o(o);  h1 = 2y
    hn = LN1(h1)  ==  LN(y) with eps/2
    out = 2 * (relu(hn @ w1 + b1) @ w2 + b2)

Sharding: data-parallel over batch, 4 per NeuronCore, no collectives.

v4: fully software-pipelined schedule; the PE queue never sees a sparse
region, so HAM stays at K=8/8 (2.4 GHz):
  pair0: conv_in -> LN0 -> q/k/v
  attn(p0) interleaved with conv_in(p1)        [filler stride 1]
  LN0(p1); conv_o(p0); conv_q(p1); LN1(p0) under conv_k/v(p1); hnT(p0)
  attn(p1) interleaved with ffn(b0)+ff1(b1)    [filler stride 3]
  conv_o(p1); ff2(b1) covers LN1(p1); hnT(p1); ffn(b2); ffn(b3)
Score matmuls head-packed via tile_position (two K=64 matmuls run
concurrently).  All matmul operands bf16 (fp32 PSUM).  Biases applied in
the evictions via pre-tiled bias tiles (no rank-1 bias matmuls).
Conv/relu evictions on the vector engine; out-DMAs alternate between the
SP and Activation DMA queues.
"""
import numpy as np
from contextlib import ExitStack

try:
    import concourse.bass as bass
except ImportError:  # pragma: no cover - path fallback for bare containers
    import sys
    for _p in ("/opt/trn_rl_repo", "/root/.axon_site/_ro/trn_rl_repo"):
        if _p not in sys.path:
            sys.path.insert(0, _p)
    import concourse.bass as bass

import concourse.mybir as mybir
import concourse.tile as tile
from concourse import bacc
from concourse.bass_utils import run_bass_kernel_spmd
from concourse.masks import make_identity

B, C, D, H, DFF, KW = 32, 512, 512, 8, 2048, 7
NCORES = 8
BL = B // NCORES          # 4 batch elements per core
DH = D // H               # 64
PAD = KW - 1              # 6
EPS = 1e-6
F32 = mybir.dt.float32
BF = mybir.dt.bfloat16
NPBF = mybir.dt.np(BF)
import os as _os
WBUFS = int(_os.environ.get("K_WBUFS", "8"))
AF = mybir.ActivationFunctionType
ALU = mybir.AluOpType


def _conv_w_host(w):
    """(cout, cin, KW) -> (4, 128, KW*512): [ci][p][k*512+cout]."""
    return np.ascontiguousarray(
        w.transpose(1, 2, 0).reshape(4, 128, KW * C)).astype(NPBF)


def build_nc(reps=1):
    nc = bacc.Bacc("TRN2", target_bir_lowering=False, debug=False)

    xp = nc.declare_dram_parameter("xp", [BL, 4, 128, PAD + D], BF, isOutput=False)
    wps = {n: nc.declare_dram_parameter(n, [4, 128, KW * C], BF, isOutput=False)
           for n in ("win", "wq", "wk", "wv", "wo")}
    w1p = nc.declare_dram_parameter("w1p", [4, 128, DFF], BF, isOutput=False)
    w2p = nc.declare_dram_parameter("w2p", [16, 128, D], BF, isOutput=False)
    bppp = nc.declare_dram_parameter("bppp", [128, 36], F32, isOutput=False)
    # pre-tiled free-dim bias rows (bias varies along the free axis)
    brt = {n: nc.declare_dram_parameter(n, [128, D], BF, isOutput=False)
           for n in ("bqt", "bkt", "b2x2t")}
    lnp = {n: nc.declare_dram_parameter(n, [128, D], BF, isOutput=False)
           for n in ("ln0g", "ln0b", "ln1g", "ln1b")}
    onesp = nc.declare_dram_parameter("onesp", [128, 4, 8, 2], BF, isOutput=False)
    zerosp = nc.declare_dram_parameter("zerosp", [128, 4, PAD], BF, isOutput=False)
    outp = nc.declare_dram_parameter("outp", [BL, C, D], F32, isOutput=True)

    with tile.TileContext(nc) as tc, ExitStack() as octx:
        cp = octx.enter_context(tc.tile_pool(name="consts", bufs=1))
        pmm = octx.enter_context(tc.tile_pool(name="pmm", bufs=4, space="PSUM"))
        patt = octx.enter_context(tc.tile_pool(name="patt", bufs=4, space="PSUM"))
        wconv = octx.enter_context(tc.tile_pool(name="wconv", bufs=WBUFS))
        act = octx.enter_context(tc.tile_pool(name="act", bufs=2))
        expp = octx.enter_context(tc.tile_pool(name="expp", bufs=6))
        lnw = octx.enter_context(tc.tile_pool(name="lnw", bufs=2))
        stat = octx.enter_context(tc.tile_pool(name="stat", bufs=16))
        rpool = octx.enter_context(tc.tile_pool(name="rpool", bufs=1))
        obp = octx.enter_context(tc.tile_pool(name="obp", bufs=2))

        # -- input / first-conv weights first: they gate the first matmul --
        def load_x(bs):
            x_t = {}
            for b in bs:
                x_t[b] = act.tile([128, 4, PAD + D], BF, tag="xyh",
                                  name=f"x{b}", bufs=4)
                nc.sync.dma_start(
                    x_t[b][:], xp.ap()[b].rearrange("c p t -> p c t"))
            return x_t

        def load_w(param, label):
            ts = []
            for ci in range(4):
                t = wconv.tile([128, KW * C], BF, tag="w", name=f"{label}{ci}")
                nc.sync.dma_start(t[:], param.ap()[ci])
                ts.append(t)
            return ts

        x0 = load_x([0, 1])
        w_in0 = load_w(wps["win"], "win0")

        def ctile(name, shape, dtype, src):
            t = cp.tile(shape, dtype, tag=name, name=name)
            nc.sync.dma_start(t[:], src)
            return t

        bpp = ctile("bpp", [128, 36], F32, bppp.ap())
        ones_t = ctile("ones", [128, 4, 8, 2], BF, onesp.ap())
        zeros_t = ctile("zeros", [128, 4, PAD], BF, zerosp.ap())
        ln_t = {n: ctile(n, [128, D], BF, lnp[n].ap()) for n in lnp}
        br_t = {n: ctile(n, [128, D], BF, brt[n].ap())
                for n in ("bqt", "bkt")}
        identb = cp.tile([128, 128], BF, tag="identb", name="identb")
        make_identity(nc, identb[:])

        # FFN weights: tiles allocated now, DMA deferred (needed ~400us in)
        w1t = [cp.tile([128, DFF], BF, tag=f"w1_{i}", name=f"w1_{i}")
               for i in range(4)]
        w2t = [cp.tile([128, D], BF, tag=f"w2_{i}", name=f"w2_{i}")
               for i in range(16)]
        hnT = {b: cp.tile([128, 4, D], BF, tag=f"hnT{b}", name=f"hnT{b}")
               for b in range(BL)}
        b2x2t = cp.tile([128, D], BF, tag="b2x2t", name="b2x2t")

        def late_dmas():
            nc.sync.dma_start(b2x2t[:], brt["b2x2t"].ap())
            for i in range(4):
                nc.sync.dma_start(w1t[i][:], w1p.ap()[i])
            for i in range(16):
                nc.sync.dma_start(w2t[i][:], w2p.ap()[i])

        def conv_std_gen(bs, wt, src, writer):
            """std conv: out[cout, t] accumulated over (cin chunk, tap);
            one yield per (ci, k) pair of batch matmuls."""
            for oc in range(4):
                ps = {b: pmm.tile([128, D], F32, tag="mm", name=f"cs{oc}{b}")
                      for b in bs}
                for ci in range(4):
                    for k in range(KW):
                        lhsT = wt[ci][:, k * C + oc * 128: k * C + oc * 128 + 128]
                        for b in bs:
                            nc.tensor.matmul(
                                ps[b][:], lhsT, src[b][:, ci, k:k + D],
                                start=(ci == 0 and k == 0),
                                stop=(ci == 3 and k == KW - 1))
                        yield True
                for b in bs:
                    writer(b, oc, ps[b])

        def drain(g):
            for _ in g:
                pass

        def conv_T(bs, wt, src, bias_t, dst):
            """transposed conv: out[t, cout]; bias added in the eviction."""
            for tcn in range(4):
                ps = {b: pmm.tile([128, D], F32, tag="mm", name=f"cT{tcn}{b}")
                      for b in bs}
                for ci in range(4):
                    for k in range(KW):
                        rhs = wt[ci][:, k * C:(k + 1) * C]
                        for b in bs:
                            lhsT = src[b][:, ci, tcn * 128 + k: tcn * 128 + k + 128]
                            nc.tensor.matmul(ps[b][:], lhsT, rhs,
                                             start=(ci == 0 and k == 0),
                                             stop=(ci == 3 and k == KW - 1))
                for b in bs:
                    nc.vector.tensor_add(dst[b][:, tcn, :], ps[b][:],
                                         bias_t[:])

        def transpose_512(src_t, dst_t, label):
            """[c-chunks, t] bf16 std tile -> [t-chunks, c] via PE."""
            for tcn in range(4):
                for cc in range(4):
                    tp = patt.tile([128, 128], BF, tag="att",
                                   name=f"tp{label}{tcn}{cc}")
                    nc.tensor.transpose(
                        tp[:], src_t[:, cc, tcn * 128:(tcn + 1) * 128],
                        identb[:])
                    nc.vector.tensor_copy(
                        dst_t[:, tcn, cc * 128:(cc + 1) * 128], tp[:])

        def emit_ln(bs, src, dst, g_t, b_t, eps, padded_src):
            for b in bs:
                for c in range(4):
                    sv = (src[b][:, c, PAD:PAD + D] if padded_src
                          else src[b][:, c, :])
                    sm = stat.tile([128, 1], F32, tag="st", name=f"sm{b}{c}")
                    nc.vector.reduce_sum(sm[:], sv, axis=mybir.AxisListType.X)
                    mn = stat.tile([128, 1], F32, tag="st", name=f"mn{b}{c}")
                    nc.scalar.mul(mn[:], sm[:], 1.0 / D)
                    cent = lnw.tile([128, D], F32, tag="lw", name=f"ce{b}{c}")
                    nc.vector.tensor_scalar(cent[:], sv, mn[:], None,
                                            op0=ALU.subtract)
                    scr = lnw.tile([128, D], F32, tag="lw", name=f"sc{b}{c}")
                    sq = stat.tile([128, 1], F32, tag="st", name=f"sq{b}{c}")
                    nc.scalar.activation(scr[:], cent[:], AF.Square,
                                         accum_out=sq[:])
                    st = stat.tile([128, 1], F32, tag="st", name=f"sd{b}{c}")
                    nc.scalar.activation(st[:], sq[:], AF.Sqrt,
                                         scale=1.0 / (D - 1))
                    dn = stat.tile([128, 1], F32, tag="st", name=f"dn{b}{c}")
                    nc.vector.tensor_scalar_add(dn[:], st[:], eps)
                    iv = stat.tile([128, 1], F32, tag="st", name=f"iv{b}{c}")
                    nc.vector.reciprocal(iv[:], dn[:])
                    tmp = lnw.tile([128, D], F32, tag="lw", name=f"tm{b}{c}")
                    nc.vector.scalar_tensor_tensor(
                        tmp[:], in0=cent[:], scalar=iv[:], in1=g_t[:],
                        op0=ALU.mult, op1=ALU.mult)
                    dv = (dst[b][:, c, PAD:PAD + D] if padded_src
                          else dst[b][:, c, :])
                    nc.vector.tensor_add(dv, tmp[:], b_t[:])

        def zero_pads(t):
            nc.scalar.copy(t[:, :, 0:PAD], zeros_t[:])

        class Filler:
            """Rations dense-work emission into sparse code regions."""
            def __init__(self, gens, stride=1):
                import itertools
                self.it = itertools.chain(*gens)
                self.stride = stride
                self.cnt = 0

            def fill(self, n=1):
                for _ in range(n):
                    self.cnt += 1
                    if self.cnt % self.stride == 0:
                        if next(self.it, None) is None:
                            return

            def drain(self):
                for _ in self.it:
                    pass

        def attn_pair(bs, qT, kT, vaug, ofull, filler):
            """attention for both b of a pair; head-packed score matmuls;
            filler.fill(1) interleaves dense work into the PE queue."""
            for tcn in range(4):
                for b in bs:
                    h0, h1 = 2 * tcn, 2 * tcn + 1
                    ops0 = [patt.tile([128, DH + 2], F32, tag="att",
                                      name=f"oa{b}{h0}{qc}")
                            for qc in range(4)]
                    ex1 = {}
                    for kc in range(4):
                        e = {}
                        for hh, prow in ((0, 0), (1, DH)):
                            sp = pmm.tile([128, D], F32, tag="mm",
                                          name=f"s{b}{tcn}{kc}{hh}")
                            nc.tensor.matmul(
                                sp[:],
                                kT[b][prow:prow + DH, tcn,
                                      kc * 128:(kc + 1) * 128],
                                qT[b][prow:prow + DH, tcn, :],
                                start=True, stop=True,
                                tile_position=(prow, 0))
                            e[hh] = expp.tile([128, D], BF, tag="e",
                                              name=f"e{b}{tcn}{kc}{hh}")
                            nc.scalar.activation(e[hh][:], sp[:], AF.Exp,
                                                 scale=1.0 / np.sqrt(DH))
                        ex1[kc] = e[1]
                        filler.fill(1)
                        for qc in range(4):
                            nc.tensor.matmul(
                                ops0[qc][:],
                                e[0][:, qc * 128:(qc + 1) * 128],
                                vaug[b][:, kc, h0, :],
                                start=(kc == 0), stop=(kc == 3))
                        filler.fill(1)
                    for qc in range(4):
                        rec = stat.tile([128, 1], F32, tag="st",
                                        name=f"ra{b}{h0}{qc}")
                        nc.vector.reciprocal(rec[:], ops0[qc][:, DH:DH + 1])
                        nc.vector.tensor_scalar_mul(
                            ofull[b][:, qc, PAD + h0 * DH:PAD + (h0 + 1) * DH],
                            ops0[qc][:, 0:DH], rec[:])
                    filler.fill(1)
                    ops1 = [patt.tile([128, DH + 2], F32, tag="att",
                                      name=f"ob{b}{h1}{qc}")
                            for qc in range(4)]
                    for kc in range(4):
                        for qc in range(4):
                            nc.tensor.matmul(
                                ops1[qc][:],
                                ex1[kc][:, qc * 128:(qc + 1) * 128],
                                vaug[b][:, kc, h1, :],
                                start=(kc == 0), stop=(kc == 3))
                        filler.fill(1)
                    for qc in range(4):
                        rec = stat.tile([128, 1], F32, tag="st",
                                        name=f"rb{b}{h1}{qc}")
                        nc.vector.reciprocal(rec[:], ops1[qc][:, DH:DH + 1])
                        nc.vector.tensor_scalar_mul(
                            ofull[b][:, qc, PAD + h1 * DH:PAD + (h1 + 1) * DH],
                            ops1[qc][:, 0:DH], rec[:])
                    filler.fill(1)

        _rl = {}

        def ffn_ff1_gen(b):
            rl = rpool.tile([128, 16, D], BF, tag="r", name=f"rl{b}")
            _rl[b] = rl
            for fc in range(16):
                ps = pmm.tile([128, D], F32, tag="mm", name=f"f{b}{fc}")
                for tcn in range(4):
                    nc.tensor.matmul(ps[:], w1t[tcn][:, fc * 128:(fc + 1) * 128],
                                     hnT[b][:, tcn, :],
                                     start=(tcn == 0), stop=(tcn == 3))
                nc.vector.tensor_scalar(rl[:, fc, :], ps[:],
                                        bpp[:, 12 + fc:13 + fc], 0.0,
                                        op0=ALU.add, op1=ALU.max)
                yield True

        def ffn_ff2_gen(b):
            rl = _rl.pop(b)
            for cc in range(4):
                ps2 = pmm.tile([128, D], F32, tag="mm", name=f"g{b}{cc}")
                for fc in range(16):
                    nc.tensor.matmul(ps2[:],
                                     rl[:, fc, cc * 128:(cc + 1) * 128],
                                     w2t[fc][:], start=(fc == 0),
                                     stop=(fc == 15))
                ob = obp.tile([128, D], F32, tag="ob", name=f"ob{b}{cc}")
                nc.vector.scalar_tensor_tensor(
                    ob[:], in0=ps2[:], scalar=2.0, in1=b2x2t[:],
                    op0=ALU.mult, op1=ALU.add)
                eng = nc.sync if cc % 2 == 0 else nc.scalar
                eng.dma_start(outp.ap()[b, cc * 128:(cc + 1) * 128, :], ob[:])
                yield True

        def ffn_gen(b):
            yield from ffn_ff1_gen(b)
            yield from ffn_ff2_gen(b)

        def conv_in_gen(bs, x_t, w_t, qin):
            def wr_qin(b, oc, ps):
                nc.vector.tensor_scalar(qin[b][:, oc, PAD:PAD + D], ps[:],
                                        bpp[:, oc:oc + 1], None, op0=ALU.add)
            return conv_std_gen(bs, w_t, x_t, wr_qin)

        def alloc_padded(bs, tag):
            out = {}
            for b in bs:
                out[b] = act.tile([128, 4, PAD + D], BF, tag=tag,
                                  name=f"{tag}{b}")
                zero_pads(out[b])
            return out

        def conv_v_stage(bs, xn):
            w_t = load_w(wps["wv"], f"wv{bs[0]}")
            vaug = {}
            for b in bs:
                vaug[b] = act.tile([128, 4, H, DH + 2], BF, tag="vaug",
                                   name=f"vaug{b}")
                nc.scalar.copy(vaug[b][:, :, :, DH:DH + 2], ones_t[:])

            def wr_v(b, oc, ps):
                nc.vector.tensor_scalar(
                    vaug[b][:, oc, :, 0:DH],
                    ps[:].rearrange("p (h dd) -> p h dd", h=H),
                    bpp[:, 4 + oc:5 + oc], None, op0=ALU.add)
            drain(conv_std_gen(bs, w_t, xn, wr_v))
            return vaug

        def conv_o_stage(bs, ofull):
            w_t = load_w(wps["wo"], f"wo{bs[0]}")
            y = {b: act.tile([128, 4, D], BF, tag="xyh", name=f"y{b}", bufs=4)
                 for b in bs}

            def wr_y(b, oc, ps):
                nc.vector.tensor_scalar(y[b][:, oc, :], ps[:],
                                        bpp[:, 8 + oc:9 + oc], None,
                                        op0=ALU.add)
            drain(conv_std_gen(bs, w_t, ofull, wr_y))
            return y

        def ln1_stage(bs, y):
            hn = {b: act.tile([128, 4, D], BF, tag="xyh", name=f"hn{b}",
                              bufs=4)
                  for b in bs}
            emit_ln(bs, y, hn, ln_t["ln1g"], ln_t["ln1b"], EPS / 2,
                    padded_src=False)
            return hn

        for _rep in range(reps):
            bs0, bs1 = [0, 1], [2, 3]
            # ---- pair0 front ----
            qin0 = alloc_padded(bs0, "qin")
            drain(conv_in_gen(bs0, x0, w_in0, qin0))
            late_dmas()          # FFN weights load in the shadow of conv_in
            xn0 = alloc_padded(bs0, "xn")
            emit_ln(bs0, qin0, xn0, ln_t["ln0g"], ln_t["ln0b"], EPS,
                    padded_src=True)
            qT0 = {b: act.tile([128, 4, D], BF, tag="qT", name=f"qT{b}")
                   for b in bs0}
            kT0 = {b: act.tile([128, 4, D], BF, tag="kT", name=f"kT{b}")
                   for b in bs0}
            conv_T(bs0, load_w(wps["wq"], "wq0"), qin0, br_t["bqt"], qT0)
            conv_T(bs0, load_w(wps["wk"], "wk0"), xn0, br_t["bkt"], kT0)
            vaug0 = conv_v_stage(bs0, xn0)

            # prefetch pair1 inputs; DMAs overlap attn(p0)
            x1 = load_x(bs1)
            w_in1 = load_w(wps["win"], "win1")

            # ---- attn(pair0) interleaved with conv_in(pair1) ----
            of0 = alloc_padded(bs0, "of")
            qin1 = alloc_padded(bs1, "qin")
            fill0 = Filler([conv_in_gen(bs1, x1, w_in1, qin1)], stride=1)
            attn_pair(bs0, qT0, kT0, vaug0, of0, fill0)
            fill0.drain()

            # LN0(p1) on vector/scalar overlaps conv_o(p0) on the PE
            xn1 = alloc_padded(bs1, "xn")
            emit_ln(bs1, qin1, xn1, ln_t["ln0g"], ln_t["ln0b"], EPS,
                    padded_src=True)
            y0 = conv_o_stage(bs0, of0)

            qT1 = {b: act.tile([128, 4, D], BF, tag="qT", name=f"qT{b}")
                   for b in bs1}
            kT1 = {b: act.tile([128, 4, D], BF, tag="kT", name=f"kT{b}")
                   for b in bs1}
            conv_T(bs1, load_w(wps["wq"], "wq1"), qin1, br_t["bqt"], qT1)
            hn0 = ln1_stage(bs0, y0)        # vector; overlaps conv_k/v(p1)
            conv_T(bs1, load_w(wps["wk"], "wk1"), xn1, br_t["bkt"], kT1)
            vaug1 = conv_v_stage(bs1, xn1)
            for b in bs0:
                transpose_512(hn0[b], hnT[b], f"h{b}")

            # ---- attn(pair1) interleaved with ffn(b0) + ff1(b1) ----
            of1 = alloc_padded(bs1, "of")
            fill1 = Filler([ffn_gen(0), ffn_ff1_gen(1)], stride=3)
            attn_pair(bs1, qT1, kT1, vaug1, of1, fill1)
            fill1.drain()

            drain(ffn_ff2_gen(1))           # ready while conv_o(p1) evicts
            y1 = conv_o_stage(bs1, of1)
            hn1 = ln1_stage(bs1, y1)
            for b in bs1:
                transpose_512(hn1[b], hnT[b], f"h{b}")
            drain(ffn_gen(2))
            drain(ffn_gen(3))

    nc.compile()
    return nc


def prep_in_maps(inputs):
    """Full inputs -> list of 8 per-core input dicts (host-side prep)."""
    f = lambda a: np.ascontiguousarray(np.asarray(a, dtype=np.float32))
    x = f(inputs["x"])
    xpad = np.zeros((B, 4, 128, PAD + D), np.float32)
    xpad[:, :, :, PAD:] = x.reshape(B, 4, 128, D)
    xpad = xpad.astype(NPBF)

    shared = {
        "win": _conv_w_host(f(inputs["w_conv_in"])),
        "wq": _conv_w_host(f(inputs["wq"])),
        "wk": _conv_w_host(f(inputs["wk"])),
        "wv": _conv_w_host(f(inputs["wv"])),
        "wo": _conv_w_host(f(inputs["wo"])),
        "w1p": f(inputs["w1"]).reshape(4, 128, DFF).astype(NPBF),
        "w2p": f(inputs["w2"]).reshape(16, 128, D).astype(NPBF),
        "bqt": np.tile(f(inputs["bq"]), (128, 1)).astype(NPBF),
        "bkt": np.tile(f(inputs["bk"]), (128, 1)).astype(NPBF),
        "b2x2t": np.tile(2.0 * f(inputs["b2"]), (128, 1)).astype(NPBF),
        "bppp": np.stack(
            [f(inputs["b_conv_in"]).reshape(4, 128)[i] for i in range(4)]
            + [f(inputs["bv"]).reshape(4, 128)[i] for i in range(4)]
            + [f(inputs["bo"]).reshape(4, 128)[i] for i in range(4)]
            + [f(inputs["b1"]).reshape(16, 128)[i] for i in range(16)]
            + [f(inputs["bq"]).reshape(4, 128)[i] for i in range(4)]
            + [f(inputs["bk"]).reshape(4, 128)[i] for i in range(4)],
            axis=1),
        "ln0g": np.tile(f(inputs["ln0_g"]), (128, 1)).astype(NPBF),
        "ln0b": np.tile(f(inputs["ln0_b"]), (128, 1)).astype(NPBF),
        "ln1g": np.tile(f(inputs["ln1_g"]), (128, 1)).astype(NPBF),
        "ln1b": np.tile(f(inputs["ln1_b"]), (128, 1)).astype(NPBF),
        "onesp": np.concatenate([np.ones((128, 4, 8, 1), np.float32),
                                 np.zeros((128, 4, 8, 1), np.float32)],
                                axis=3).astype(NPBF),
        "zerosp": np.zeros((128, 4, PAD), NPBF),
    }
    shared = {k: np.ascontiguousarray(v) for k, v in shared.items()}
    return [dict(shared, xp=np.ascontiguousarray(xpad[c * BL:(c + 1) * BL]))
            for c in range(NCORES)]


_NC_CACHE = {}


def get_nc(reps=1):
    if reps not in _NC_CACHE:
        _NC_CACHE[reps] = build_nc(reps)
    return _NC_CACHE[reps]


def kernel(**inputs) -> np.ndarray:
    nc = get_nc()
    in_maps = prep_in_maps(inputs)
    res = run_bass_kernel_spmd(nc, in_maps, list(range(NCORES)))
    return np.concatenate([res.results[c]["outp"] for c in range(NCORES)],
                          axis=0).astype(np.float32)


# revision 10
# speedup vs baseline: 1.0075x; 1.0075x over previous
"""TRN2 Bass kernel for nn_CNN_transformer_hr_xyz_41051297415299.

Reference model (B=32, C=512, D=512, H=8, DFF=2048, K=7), per batch element:
    query_in = causal_conv_in(x)                 # conv over last axis t, mixing C
    xn       = LN0(query_in)                     # over t, (x-m)/(std+eps), ddof=1
    q = conv_q(query_in); k = conv_k(xn); v = conv_v(xn)
    heads split the t axis (8 x 64); attention over the C axis
    o  = softmax(q k^T / 8) v   -> (C, D)
    y  = conv_o(o);  h1 = 2y
    hn = LN1(h1)  ==  LN(y) with eps/2
    out = 2 * (relu(hn @ w1 + b1) @ w2 + b2)

Sharding: data-parallel over batch, 4 per NeuronCore, no collectives.

v4: fully software-pipelined schedule; the PE queue never sees a sparse
region, so HAM stays at K=8/8 (2.4 GHz):
  pair0: conv_in -> LN0 -> q/k/v
  attn(p0) interleaved with conv_in(p1)        [filler stride 1]
  LN0(p1); conv_o(p0); conv_q(p1); LN1(p0) under conv_k/v(p1); hnT(p0)
  attn(p1) interleaved with ffn(b0)+ff1(b1)    [filler stride 3]
  conv_o(p1); ff2(b1) covers LN1(p1); hnT(p1); ffn(b2); ffn(b3)
Score matmuls head-packed via tile_position (two K=64 matmuls run
concurrently).  All matmul operands bf16 (fp32 PSUM).  Biases applied in
the evictions via pre-tiled bias tiles (no rank-1 bias matmuls).
Conv/relu evictions on the vector engine; out-DMAs alternate between the
SP and Activation DMA queues.
"""
import numpy as np
from contextlib import ExitStack

try:
    import concourse.bass as bass
except ImportError:  # pragma: no cover - path fallback for bare containers
    import sys
    for _p in ("/opt/trn_rl_repo", "/root/.axon_site/_ro/trn_rl_repo"):
        if _p not in sys.path:
            sys.path.insert(0, _p)
    import concourse.bass as bass

import concourse.mybir as mybir
import concourse.tile as tile
from concourse import bacc
from concourse.bass_utils import run_bass_kernel_spmd
from concourse.masks import make_identity

B, C, D, H, DFF, KW = 32, 512, 512, 8, 2048, 7
NCORES = 8
BL = B // NCORES          # 4 batch elements per core
DH = D // H               # 64
PAD = KW - 1              # 6
EPS = 1e-6
F32 = mybir.dt.float32
BF = mybir.dt.bfloat16
NPBF = mybir.dt.np(BF)
import os as _os
WBUFS = int(_os.environ.get("K_WBUFS", "7"))
AF = mybir.ActivationFunctionType
ALU = mybir.AluOpType


def _conv_w_host(w):
    """(cout, cin, KW) -> (4, 128, KW*512): [ci][p][k*512+cout]."""
    return np.ascontiguousarray(
        w.transpose(1, 2, 0).reshape(4, 128, KW * C)).astype(NPBF)


def build_nc(reps=1):
    nc = bacc.Bacc("TRN2", target_bir_lowering=False, debug=False)

    xp = nc.declare_dram_parameter("xp", [BL, 4, 128, PAD + D], BF, isOutput=False)
    wps = {n: nc.declare_dram_parameter(n, [4, 128, KW * C], BF, isOutput=False)
           for n in ("win", "wq", "wk", "wv", "wo")}
    w1p = nc.declare_dram_parameter("w1p", [4, 128, DFF], BF, isOutput=False)
    w2p = nc.declare_dram_parameter("w2p", [16, 128, D], BF, isOutput=False)
    bppp = nc.declare_dram_parameter("bppp", [128, 36], F32, isOutput=False)
    # pre-tiled free-dim bias rows (bias varies along the free axis)
    brt = {n: nc.declare_dram_parameter(n, [128, D], BF, isOutput=False)
           for n in ("bqt", "bkt", "b2x2t")}
    lnp = {n: nc.declare_dram_parameter(n, [128, D], BF, isOutput=False)
           for n in ("ln0g", "ln0b", "ln1g", "ln1b")}
    onesp = nc.declare_dram_parameter("onesp", [128, 4, 8, 2], BF, isOutput=False)
    zerosp = nc.declare_dram_parameter("zerosp", [128, 4, PAD], BF, isOutput=False)
    outp = nc.declare_dram_parameter("outp", [BL, C, D], F32, isOutput=True)

    with tile.TileContext(nc) as tc, ExitStack() as octx:
        cp = octx.enter_context(tc.tile_pool(name="consts", bufs=1))
        pmm = octx.enter_context(tc.tile_pool(name="pmm", bufs=4, space="PSUM"))
        patt = octx.enter_context(tc.tile_pool(name="patt", bufs=4, space="PSUM"))
        wconv = octx.enter_context(tc.tile_pool(name="wconv", bufs=WBUFS))
        act = octx.enter_context(tc.tile_pool(name="act", bufs=2))
        expp = octx.enter_context(tc.tile_pool(name="expp", bufs=6))
        lnw = octx.enter_context(tc.tile_pool(name="lnw", bufs=2))
        stat = octx.enter_context(tc.tile_pool(name="stat", bufs=16))
        rpool = octx.enter_context(tc.tile_pool(name="rpool", bufs=1))
        obp = octx.enter_context(tc.tile_pool(name="obp", bufs=2))

        # -- input / first-conv weights first: they gate the first matmul --
        def load_x(bs):
            x_t = {}
            for b in bs:
                x_t[b] = act.tile([128, 4, PAD + D], BF, tag="xyh",
                                  name=f"x{b}", bufs=4)
                nc.sync.dma_start(
                    x_t[b][:], xp.ap()[b].rearrange("c p t -> p c t"))
            return x_t

        def load_w(param, label):
            ts = []
            for ci in range(4):
                t = wconv.tile([128, KW * C], BF, tag="w", name=f"{label}{ci}")
                nc.sync.dma_start(t[:], param.ap()[ci])
                ts.append(t)
            return ts

        x0 = load_x([0, 1])
        w_in0 = load_w(wps["win"], "win0")

        def ctile(name, shape, dtype, src):
            t = cp.tile(shape, dtype, tag=name, name=name)
            nc.sync.dma_start(t[:], src)
            return t

        bpp = ctile("bpp", [128, 36], F32, bppp.ap())
        ones_t = ctile("ones", [128, 4, 8, 2], BF, onesp.ap())
        zeros_t = ctile("zeros", [128, 4, PAD], BF, zerosp.ap())
        ln_t = {n: ctile(n, [128, D], BF, lnp[n].ap()) for n in lnp}
        br_t = {n: ctile(n, [128, D], BF, brt[n].ap())
                for n in ("bqt", "bkt")}
        identb = cp.tile([128, 128], BF, tag="identb", name="identb")
        make_identity(nc, identb[:])

        # FFN weights: tiles allocated now, DMA deferred (needed ~400us in)
        w1t = [cp.tile([128, DFF], BF, tag=f"w1_{i}", name=f"w1_{i}")
               for i in range(4)]
        w2t = [cp.tile([128, D], BF, tag=f"w2_{i}", name=f"w2_{i}")
               for i in range(16)]
        hnT = {b: cp.tile([128, 4, D], BF, tag=f"hnT{b}", name=f"hnT{b}")
               for b in range(BL)}
        b2x2t = cp.tile([128, D], BF, tag="b2x2t", name="b2x2t")

        def late_dmas():
            nc.sync.dma_start(b2x2t[:], brt["b2x2t"].ap())
            for i in range(4):
                nc.sync.dma_start(w1t[i][:], w1p.ap()[i])
            for i in range(16):
                nc.sync.dma_start(w2t[i][:], w2p.ap()[i])

        def conv_std_gen(bs, wt, src, writer):
            """std conv: out[cout, t] accumulated over (cin chunk, tap);
            one yield per (ci, k) pair of batch matmuls."""
            for oc in range(4):
                ps = {b: pmm.tile([128, D], F32, tag="mm", name=f"cs{oc}{b}")
                      for b in bs}
                for ci in range(4):
                    for k in range(KW):
                        lhsT = wt[ci][:, k * C + oc * 128: k * C + oc * 128 + 128]
                        for b in bs:
                            nc.tensor.matmul(
                                ps[b][:], lhsT, src[b][:, ci, k:k + D],
                                start=(ci == 0 and k == 0),
                                stop=(ci == 3 and k == KW - 1))
                        yield True
                for b in bs:
                    writer(b, oc, ps[b])

        def drain(g):
            for _ in g:
                pass

        def conv_T(bs, wt, src, bias_t, dst):
            """transposed conv: out[t, cout]; bias added in the eviction."""
            for tcn in range(4):
                ps = {b: pmm.tile([128, D], F32, tag="mm", name=f"cT{tcn}{b}")
                      for b in bs}
                for ci in range(4):
                    for k in range(KW):
                        rhs = wt[ci][:, k * C:(k + 1) * C]
                        for b in bs:
                            lhsT = src[b][:, ci, tcn * 128 + k: tcn * 128 + k + 128]
                            nc.tensor.matmul(ps[b][:], lhsT, rhs,
                                             start=(ci == 0 and k == 0),
                                             stop=(ci == 3 and k == KW - 1))
                for b in bs:
                    nc.vector.tensor_add(dst[b][:, tcn, :], ps[b][:],
                                         bias_t[:])

        def transpose_512(src_t, dst_t, label):
            """[c-chunks, t] bf16 std tile -> [t-chunks, c] via PE."""
            for tcn in range(4):
                for cc in range(4):
                    tp = patt.tile([128, 128], BF, tag="att",
                                   name=f"tp{label}{tcn}{cc}")
                    nc.tensor.transpose(
                        tp[:], src_t[:, cc, tcn * 128:(tcn + 1) * 128],
                        identb[:])
                    nc.vector.tensor_copy(
                        dst_t[:, tcn, cc * 128:(cc + 1) * 128], tp[:])

        def emit_ln(bs, src, dst, g_t, b_t, eps, padded_src):
            for b in bs:
                for c in range(4):
                    sv = (src[b][:, c, PAD:PAD + D] if padded_src
                          else src[b][:, c, :])
                    sm = stat.tile([128, 1], F32, tag="st", name=f"sm{b}{c}")
                    nc.vector.reduce_sum(sm[:], sv, axis=mybir.AxisListType.X)
                    mn = stat.tile([128, 1], F32, tag="st", name=f"mn{b}{c}")
                    nc.scalar.mul(mn[:], sm[:], 1.0 / D)
                    cent = lnw.tile([128, D], F32, tag="lw", name=f"ce{b}{c}")
                    nc.vector.tensor_scalar(cent[:], sv, mn[:], None,
                                            op0=ALU.subtract)
                    scr = lnw.tile([128, D], F32, tag="lw", name=f"sc{b}{c}")
                    sq = stat.tile([128, 1], F32, tag="st", name=f"sq{b}{c}")
                    nc.scalar.activation(scr[:], cent[:], AF.Square,
                                         accum_out=sq[:])
                    st = stat.tile([128, 1], F32, tag="st", name=f"sd{b}{c}")
                    nc.scalar.activation(st[:], sq[:], AF.Sqrt,
                                         scale=1.0 / (D - 1))
                    dn = stat.tile([128, 1], F32, tag="st", name=f"dn{b}{c}")
                    nc.vector.tensor_scalar_add(dn[:], st[:], eps)
                    iv = stat.tile([128, 1], F32, tag="st", name=f"iv{b}{c}")
                    nc.vector.reciprocal(iv[:], dn[:])
                    tmp = lnw.tile([128, D], F32, tag="lw", name=f"tm{b}{c}")
                    nc.vector.scalar_tensor_tensor(
                        tmp[:], in0=cent[:], scalar=iv[:], in1=g_t[:],
                        op0=ALU.mult, op1=ALU.mult)
                    dv = (dst[b][:, c, PAD:PAD + D] if padded_src
                          else dst[b][:, c, :])
                    nc.vector.tensor_add(dv, tmp[:], b_t[:])

        def zero_pads(t):
            nc.scalar.copy(t[:, :, 0:PAD], zeros_t[:])

        class Filler:
            """Rations dense-work emission into sparse code regions."""
            def __init__(self, gens, stride=1):
                import itertools
                self.it = itertools.chain(*gens)
                self.stride = stride
                self.cnt = 0

            def fill(self, n=1):
                for _ in range(n):
                    self.cnt += 1
                    if self.cnt % self.stride == 0:
                        if next(self.it, None) is None:
                            return

            def drain(self):
                for _ in self.it:
                    pass

        def attn_pair(bs, qT, kT, vaug, ofull, filler):
            """attention for both b of a pair; head-packed score matmuls;
            filler.fill(1) interleaves dense work into the PE queue."""
            for tcn in range(4):
                for b in bs:
                    h0, h1 = 2 * tcn, 2 * tcn + 1
                    ops0 = [patt.tile([128, DH + 2], F32, tag="att",
                                      name=f"oa{b}{h0}{qc}")
                            for qc in range(4)]
                    ex1 = {}
                    for kc in range(4):
                        e = {}
                        for hh, prow in ((0, 0), (1, DH)):
                            sp = pmm.tile([128, D], F32, tag="mm",
                                          name=f"s{b}{tcn}{kc}{hh}")
                            nc.tensor.matmul(
                                sp[:],
                                kT[b][prow:prow + DH, tcn,
                                      kc * 128:(kc + 1) * 128],
                                qT[b][prow:prow + DH, tcn, :],
                                start=True, stop=True,
                                tile_position=(prow, 0))
                            e[hh] = expp.tile([128, D], BF, tag="e",
                                              name=f"e{b}{tcn}{kc}{hh}")
                            nc.scalar.activation(e[hh][:], sp[:], AF.Exp,
                                                 scale=1.0 / np.sqrt(DH))
                        ex1[kc] = e[1]
                        filler.fill(1)
                        for qc in range(4):
                            nc.tensor.matmul(
                                ops0[qc][:],
                                e[0][:, qc * 128:(qc + 1) * 128],
                                vaug[b][:, kc, h0, :],
                                start=(kc == 0), stop=(kc == 3))
                        filler.fill(1)
                    for qc in range(4):
                        rec = stat.tile([128, 1], F32, tag="st",
                                        name=f"ra{b}{h0}{qc}")
                        nc.vector.reciprocal(rec[:], ops0[qc][:, DH:DH + 1])
                        nc.vector.tensor_scalar_mul(
                            ofull[b][:, qc, PAD + h0 * DH:PAD + (h0 + 1) * DH],
                            ops0[qc][:, 0:DH], rec[:])
                    filler.fill(1)
                    ops1 = [patt.tile([128, DH + 2], F32, tag="att",
                                      name=f"ob{b}{h1}{qc}")
                            for qc in range(4)]
                    for kc in range(4):
                        for qc in range(4):
                            nc.tensor.matmul(
                                ops1[qc][:],
                                ex1[kc][:, qc * 128:(qc + 1) * 128],
                                vaug[b][:, kc, h1, :],
                                start=(kc == 0), stop=(kc == 3))
                        filler.fill(1)
                    for qc in range(4):
                        rec = stat.tile([128, 1], F32, tag="st",
                                        name=f"rb{b}{h1}{qc}")
                        nc.vector.reciprocal(rec[:], ops1[qc][:, DH:DH + 1])
                        nc.vector.tensor_scalar_mul(
                            ofull[b][:, qc, PAD + h1 * DH:PAD + (h1 + 1) * DH],
                            ops1[qc][:, 0:DH], rec[:])
                    filler.fill(1)

        _rl = {}

        def ffn_ff1_gen(b):
            rl = rpool.tile([128, 16, D], BF, tag="r", name=f"rl{b}")
            _rl[b] = rl
            for fc in range(16):
                ps = pmm.tile([128, D], F32, tag="mm", name=f"f{b}{fc}")
                for tcn in range(4):
                    nc.tensor.matmul(ps[:], w1t[tcn][:, fc * 128:(fc + 1) * 128],
                                     hnT[b][:, tcn, :],
                                     start=(tcn == 0), stop=(tcn == 3))
                nc.vector.tensor_scalar(rl[:, fc, :], ps[:],
                                        bpp[:, 12 + fc:13 + fc], 0.0,
                                        op0=ALU.add, op1=ALU.max)
                yield True

        def ffn_ff2_gen(b):
            rl = _rl.pop(b)
            for cc in range(4):
                ps2 = pmm.tile([128, D], F32, tag="mm", name=f"g{b}{cc}")
                for fc in range(16):
                    nc.tensor.matmul(ps2[:],
                                     rl[:, fc, cc * 128:(cc + 1) * 128],
                                     w2t[fc][:], start=(fc == 0),
                                     stop=(fc == 15))
                ob = obp.tile([128, D], F32, tag="ob", name=f"ob{b}{cc}")
                nc.vector.scalar_tensor_tensor(
                    ob[:], in0=ps2[:], scalar=2.0, in1=b2x2t[:],
                    op0=ALU.mult, op1=ALU.add)
                eng = nc.sync if cc % 2 == 0 else nc.scalar
                eng.dma_start(outp.ap()[b, cc * 128:(cc + 1) * 128, :], ob[:])
                yield True

        def ffn_gen(b):
            yield from ffn_ff1_gen(b)
            yield from ffn_ff2_gen(b)

        def conv_in_gen(bs, x_t, w_t, qin):
            def wr_qin(b, oc, ps):
                nc.vector.tensor_scalar(qin[b][:, oc, PAD:PAD + D], ps[:],
                                        bpp[:, oc:oc + 1], None, op0=ALU.add)
            return conv_std_gen(bs, w_t, x_t, wr_qin)

        def alloc_padded(bs, tag):
            out = {}
            for b in bs:
                out[b] = act.tile([128, 4, PAD + D], BF, tag=tag,
                                  name=f"{tag}{b}")
                zero_pads(out[b])
            return out

        def conv_v_stage(bs, xn):
            w_t = load_w(wps["wv"], f"wv{bs[0]}")
            vaug = {}
            for b in bs:
                vaug[b] = act.tile([128, 4, H, DH + 2], BF, tag="vaug",
                                   name=f"vaug{b}")
                nc.scalar.copy(vaug[b][:, :, :, DH:DH + 2], ones_t[:])

            def wr_v(b, oc, ps):
                nc.vector.tensor_scalar(
                    vaug[b][:, oc, :, 0:DH],
                    ps[:].rearrange("p (h dd) -> p h dd", h=H),
                    bpp[:, 4 + oc:5 + oc], None, op0=ALU.add)
            drain(conv_std_gen(bs, w_t, xn, wr_v))
            return vaug

        def conv_o_stage(bs, ofull):
            w_t = load_w(wps["wo"], f"wo{bs[0]}")
            y = {b: act.tile([128, 4, D], BF, tag="xyh", name=f"y{b}", bufs=4)
                 for b in bs}

            def wr_y(b, oc, ps):
                nc.vector.tensor_scalar(y[b][:, oc, :], ps[:],
                                        bpp[:, 8 + oc:9 + oc], None,
                                        op0=ALU.add)
            drain(conv_std_gen(bs, w_t, ofull, wr_y))
            return y

        def ln1_stage(bs, y):
            hn = {b: act.tile([128, 4, D], BF, tag="xyh", name=f"hn{b}",
                              bufs=4)
                  for b in bs}
            emit_ln(bs, y, hn, ln_t["ln1g"], ln_t["ln1b"], EPS / 2,
                    padded_src=False)
            return hn

        for _rep in range(reps):
            bs0, bs1 = [0, 1], [2, 3]
            # ---- pair0 front ----
            qin0 = alloc_padded(bs0, "qin")
            drain(conv_in_gen(bs0, x0, w_in0, qin0))
            late_dmas()          # FFN weights load in the shadow of conv_in
            xn0 = alloc_padded(bs0, "xn")
            emit_ln(bs0, qin0, xn0, ln_t["ln0g"], ln_t["ln0b"], EPS,
                    padded_src=True)
            qT0 = {b: act.tile([128, 4, D], BF, tag="qT", name=f"qT{b}")
                   for b in bs0}
            kT0 = {b: act.tile([128, 4, D], BF, tag="kT", name=f"kT{b}")
                   for b in bs0}
            conv_T(bs0, load_w(wps["wq"], "wq0"), qin0, br_t["bqt"], qT0)
            conv_T(bs0, load_w(wps["wk"], "wk0"), xn0, br_t["bkt"], kT0)
            vaug0 = conv_v_stage(bs0, xn0)

            # prefetch pair1 inputs; DMAs overlap attn(p0)
            x1 = load_x(bs1)
            w_in1 = load_w(wps["win"], "win1")

            # ---- attn(pair0) interleaved with conv_in(pair1) ----
            of0 = alloc_padded(bs0, "of")
            qin1 = alloc_padded(bs1, "qin")
            fill0 = Filler([conv_in_gen(bs1, x1, w_in1, qin1)], stride=1)
            attn_pair(bs0, qT0, kT0, vaug0, of0, fill0)
            fill0.drain()

            # LN0(p1) on vector/scalar overlaps conv_o(p0) on the PE
            xn1 = alloc_padded(bs1, "xn")
            emit_ln(bs1, qin1, xn1, ln_t["ln0g"], ln_t["ln0b"], EPS,
                    padded_src=True)
            y0 = conv_o_stage(bs0, of0)

            qT1 = {b: act.tile([128, 4, D], BF, tag="qT", name=f"qT{b}")
                   for b in bs1}
            kT1 = {b: act.tile([128, 4, D], BF, tag="kT", name=f"kT{b}")
                   for b in bs1}
            conv_T(bs1, load_w(wps["wq"], "wq1"), qin1, br_t["bqt"], qT1)
            hn0 = ln1_stage(bs0, y0)        # vector; overlaps conv_k/v(p1)
            conv_T(bs1, load_w(wps["wk"], "wk1"), xn1, br_t["bkt"], kT1)
            vaug1 = conv_v_stage(bs1, xn1)
            for b in bs0:
                transpose_512(hn0[b], hnT[b], f"h{b}")

            # ---- attn(pair1) interleaved with ffn(b0) + ff1(b1) ----
            of1 = alloc_padded(bs1, "of")
            fill1 = Filler([ffn_gen(0), ffn_ff1_gen(1)], stride=3)
            attn_pair(bs1, qT1, kT1, vaug1, of1, fill1)
            fill1.drain()

            y1 = conv_o_stage(bs1, of1)
            drain(ffn_ff2_gen(1))           # dense work under LN1(p1)
            hn1 = ln1_stage(bs1, y1)
            for b in bs1:
                transpose_512(hn1[b], hnT[b], f"h{b}")
            drain(ffn_gen(2))
            drain(ffn_gen(3))

    nc.compile()
    return nc


def prep_in_maps(inputs):
    """Full inputs -> list of 8 per-core input dicts (host-side prep)."""
    f = lambda a: np.ascontiguousarray(np.asarray(a, dtype=np.float32))
    x = f(inputs["x"])
    xpad = np.zeros((B, 4, 128, PAD + D), np.float32)
    xpad[:, :, :, PAD:] = x.reshape(B, 4, 128, D)
    xpad = xpad.astype(NPBF)

    shared = {
        "win": _conv_w_host(f(inputs["w_conv_in"])),
        "wq": _conv_w_host(f(inputs["wq"])),
        "wk": _conv_w_host(f(inputs["wk"])),
        "wv": _conv_w_host(f(inputs["wv"])),
        "wo": _conv_w_host(f(inputs["wo"])),
        "w1p": f(inputs["w1"]).reshape(4, 128, DFF).astype(NPBF),
        "w2p": f(inputs["w2"]).reshape(16, 128, D).astype(NPBF),
        "bqt": np.tile(f(inputs["bq"]), (128, 1)).astype(NPBF),
        "bkt": np.tile(f(inputs["bk"]), (128, 1)).astype(NPBF),
        "b2x2t": np.tile(2.0 * f(inputs["b2"]), (128, 1)).astype(NPBF),
        "bppp": np.stack(
            [f(inputs["b_conv_in"]).reshape(4, 128)[i] for i in range(4)]
            + [f(inputs["bv"]).reshape(4, 128)[i] for i in range(4)]
            + [f(inputs["bo"]).reshape(4, 128)[i] for i in range(4)]
            + [f(inputs["b1"]).reshape(16, 128)[i] for i in range(16)]
            + [f(inputs["bq"]).reshape(4, 128)[i] for i in range(4)]
            + [f(inputs["bk"]).reshape(4, 128)[i] for i in range(4)],
            axis=1),
        "ln0g": np.tile(f(inputs["ln0_g"]), (128, 1)).astype(NPBF),
        "ln0b": np.tile(f(inputs["ln0_b"]), (128, 1)).astype(NPBF),
        "ln1g": np.tile(f(inputs["ln1_g"]), (128, 1)).astype(NPBF),
        "ln1b": np.tile(f(inputs["ln1_b"]), (128, 1)).astype(NPBF),
        "onesp": np.concatenate([np.ones((128, 4, 8, 1), np.float32),
                                 np.zeros((128, 4, 8, 1), np.float32)],
                                axis=3).astype(NPBF),
        "zerosp": np.zeros((128, 4, PAD), NPBF),
    }
    shared = {k: np.ascontiguousarray(v) for k, v in shared.items()}
    return [dict(shared, xp=np.ascontiguousarray(xpad[c * BL:(c + 1) * BL]))
            for c in range(NCORES)]


_NC_CACHE = {}


def get_nc(reps=1):
    if reps not in _NC_CACHE:
        _NC_CACHE[reps] = build_nc(reps)
    return _NC_CACHE[reps]


def kernel(**inputs) -> np.ndarray:
    nc = get_nc()
    in_maps = prep_in_maps(inputs)
    res = run_bass_kernel_spmd(nc, in_maps, list(range(NCORES)))
    return np.concatenate([res.results[c]["outp"] for c in range(NCORES)],
                          axis=0).astype(np.float32)
